# revision 44
# baseline (speedup 1.0000x reference)
"""Trainium2 Bass kernel for nn_GemNetOutput (segment_reduce + FiLM + MLP head).

Reference computation (all fp32):
    g     = segment_sum(x, batch, num_segments=B)        # [B, H]
    gamma = domain_emb @ gamma_w.T + gamma_b             # [B, H]
    beta  = domain_emb @ beta_w.T  + beta_b              # [B, H]
    g     = gamma * g + beta
    h     = silu(g @ w1.T + b1)                          # [B, H]
    h     = silu(h @ w2.T + b2)                          # [B, H/2]
    out   = (h @ w3.T + b3).squeeze(-1)                  # [B]

Shapes: N=1e6 nodes, B=16384 graphs, H=512, FD=16.  `batch` is SORTED.

Strategy (8 NeuronCores, no collectives needed):
  - Shard by SEGMENT range: core c owns segments [c*2048, (c+1)*2048).  Since
    `batch` is sorted, each core's nodes are one contiguous slice of x.
  - Per core, process 16 windows of 128 segments.  For each window the host
    pads the window's node rows to a fixed tile count T (sentinel batch ids
    mask the padding), so the device program is fully static.
  - segment_sum on the PE: for each 128-node tile, build a one-hot
    [node, seg-in-window] matrix on-chip (tensor_scalar is_equal against an
    iota row) and accumulate matmuls into a PSUM [128 seg, 512] tile.
  - x is converted to fp8 e3m4 on the host (quarter of the f32 HBM read; the
    one-hot matmul accumulates in fp32 so only the input rounding matters;
    measured end-to-end rel err ~1.5e-2 vs the 2e-2 gate).
  - x DMA layout: partition p holds rows [p*xt, (p+1)*xt) of each block, so
    every partition line is one contiguous 4KB read (max DMA efficiency).
    The host permutes the batch-relative id stream (brt) to match.
  - One-hot tiles are bf16 (is_equal inputs/outputs 2-byte => DVE 2x/4x
    perf modes); matmul mixes bf16 stationary with fp8 moving.
  - FiLM + MLP run per-window on-device in transposed [feature, seg] layout
    (PE transpose), with biases folded in via per-partition activation bias.
"""

import sys
from contextlib import ExitStack

for _p in ("/opt/trn_rl_repo", "/opt/pypackages"):
    if _p not in sys.path:
        sys.path.append(_p)

import ml_dtypes
import numpy as np

import concourse.bass as bass
import concourse.tile as tile
from concourse import bacc, mybir
from concourse import bass_utils

dt = mybir.dt

# Problem constants (hardcoded per the contract).
N_NODES = 1_000_000
B_SEGS = 16_384
H = 512
H2 = 256
FD = 16
N_CORES = 8
SEG_W = 128          # segments per window (PSUM partition dim)

BF16 = ml_dtypes.bfloat16
F8E3 = ml_dtypes.float8_e3m4

# CoreSim has no Silu LUT; compose silu = z * sigmoid(z) when True (sim tests).
SILU_COMPOSE = False

# Blocks (by index) whose one-hot builds run on GPSIMD instead of DVE.
# (Measured: GPSIMD tensor_scalar is ~1.7us per [128,64] build -- useless.)
GPS_BLOCKS = ()


def _quad_blocks(nblocks: int, plan) -> tuple:
    """Blocks whose rows arrive as groups of 4 same-segment rows and are
    pre-summed on the DVE (4 source tiles -> 1 matmul tile).  We use the
    three blocks before the last; the last block catches the leftover
    (non-quadded) rows.  Merged tiles reuse each block's static one-hot
    plan (the straddling block's merged one-hots are just M=128)."""
    if nblocks < 5:
        return ()
    return (nblocks - 4, nblocks - 3, nblocks - 2)


def _block_plan(nblocks: int):
    """Static per-block one-hot plan: list of (m, base), plus PE block order.

    Block b of a window holds the b-th 1/nblocks slice of the window's
    (segment-sorted) rows, so its segments concentrate around
    [SEG_W*b/nblocks, SEG_W*(b+1)/nblocks).  Matmul outputs may only start
    at PSUM partition 0 or 64 for a 64-wide write, so blocks safely inside
    [0,64) or [64,128) get narrow M=64 one-hots; straddling blocks stay
    M=128.  The first M=128 block is processed FIRST on the PE: its initial
    matmul carries the start flag and covers/initializes all 128 PSUM
    partitions.  The host verifies the plan exactly against the data and
    falls back to all-M=128 if it doesn't fit.
    """
    eb = SEG_W / nblocks
    margin = 6.0  # segments of slack vs ~1.3-sigma block-edge fluctuation
    plan = []
    for b in range(nblocks):
        lo, hi = eb * b, eb * (b + 1)
        if hi + margin <= 64:
            plan.append((64, 0))
        elif lo - margin >= 64:
            plan.append((64, 64))
        else:
            plan.append((128, 0))
    if not any(m == 128 for m, _ in plan):
        plan[0] = (128, 0)
    first = next(b for b in range(nblocks) if plan[b][0] == 128)
    order = [first] + [b for b in range(nblocks) if b != first]
    return plan, order


def build_program(spc: int, t_tiles: int, xt: int, n_cores: int,
                  narrow: bool = True, quads: bool = True):
    """Build the per-core Bass/Tile program.

    spc: segments per core (multiple of 128)
    t_tiles: node tiles (of 128) per 128-segment window, multiple of xt
    xt: node subtiles per x DMA
    narrow: most blocks use M=64 one-hots at static segment bases
    quads: two blocks per window carry 4-row same-segment groups that the
        DVE pre-sums 4:1 before the PE matmul
    """
    windows = spc // SEG_W
    npw = SEG_W * t_tiles          # padded nodes per window
    npad = windows * npw           # padded nodes per core
    x_dt = dt.float8e3
    m_dt = dt.bfloat16             # MLP/film matmul + one-hot dtype

    nc = bacc.Bacc(
        "TRN2",
        target_bir_lowering=False,
        debug=False,
        enable_asserts=False,
        num_devices=n_cores,
    )

    xp = nc.dram_tensor("xp", [npad, H], x_dt, kind="ExternalInput").ap()
    brt = nc.dram_tensor("brt", [windows, 128, t_tiles], dt.float32, kind="ExternalInput").ap()
    dombT = nc.dram_tensor("dombT", [FD + 1, spc], m_dt, kind="ExternalInput").ap()
    gw = nc.dram_tensor("gw", [FD + 1, H], m_dt, kind="ExternalInput").ap()
    bw = nc.dram_tensor("bw", [FD + 1, H], m_dt, kind="ExternalInput").ap()
    w1t = nc.dram_tensor("w1t", [H, H], m_dt, kind="ExternalInput").ap()
    w2t = nc.dram_tensor("w2t", [H, H2], m_dt, kind="ExternalInput").ap()
    w3c = nc.dram_tensor("w3c", [128, H2 // 128], m_dt, kind="ExternalInput").ap()
    b1c = nc.dram_tensor("b1c", [128, H // 128], dt.float32, kind="ExternalInput").ap()
    b2c = nc.dram_tensor("b2c", [128, H2 // 128], dt.float32, kind="ExternalInput").ap()
    b3c = nc.dram_tensor("b3c", [1, 1], dt.float32, kind="ExternalInput").ap()
    iden = nc.dram_tensor("iden", [128, 128], m_dt, kind="ExternalInput").ap()
    iotr = nc.dram_tensor("iotr", [128, 128], m_dt, kind="ExternalInput").ap()
    iotf = nc.dram_tensor("iotf", [128, 128], dt.float32, kind="ExternalInput").ap()
    out = nc.dram_tensor("out", [1, spc], dt.float32, kind="ExternalOutput").ap()

    HC = H // 128       # 4 h-chunks
    JC = H // 128       # 4 layer-1 output chunks
    KC = H2 // 128      # 2 layer-2 output chunks

    with tile.TileContext(nc) as tc, ExitStack() as ctx:
        cpool = ctx.enter_context(tc.tile_pool(name="consts", bufs=1))
        xpool = ctx.enter_context(tc.tile_pool(name="x", bufs=10))
        bpool = ctx.enter_context(tc.tile_pool(name="brt", bufs=2))
        ohpool = ctx.enter_context(tc.tile_pool(name="oh", bufs=8))
        qpool = ctx.enter_context(tc.tile_pool(name="quad", bufs=6))
        spool = ctx.enter_context(tc.tile_pool(name="work", bufs=2))
        pg = ctx.enter_context(tc.tile_pool(name="pg", bufs=3, space=bass.MemorySpace.PSUM))
        pt = ctx.enter_context(tc.tile_pool(name="pt", bufs=2, space=bass.MemorySpace.PSUM))
        pm = ctx.enter_context(tc.tile_pool(name="pm", bufs=2, space=bass.MemorySpace.PSUM))

        # ---- constants / weights into SBUF ----
        iden_sb = cpool.tile([128, 128], m_dt)
        nc.sync.dma_start(iden_sb[:], iden)
        iotr_sb = cpool.tile([128, 128], m_dt)
        nc.sync.dma_start(iotr_sb[:], iotr)
        iotf_sb = cpool.tile([128, 128], dt.float32)
        nc.sync.dma_start(iotf_sb[:], iotf)
        w1_sb = cpool.tile([128, HC, H], m_dt)
        nc.sync.dma_start(w1_sb[:], w1t.rearrange("(c p) j -> p c j", p=128))
        w2_sb = cpool.tile([128, HC, H2], m_dt)
        nc.sync.dma_start(w2_sb[:], w2t.rearrange("(c p) j -> p c j", p=128))
        w3_sb = cpool.tile([128, KC], m_dt)
        nc.sync.dma_start(w3_sb[:], w3c)
        b1_sb = cpool.tile([128, JC], dt.float32)
        nc.sync.dma_start(b1_sb[:], b1c)
        b2_sb = cpool.tile([128, KC], dt.float32)
        nc.sync.dma_start(b2_sb[:], b2c)
        b3_sb = cpool.tile([1, 1], dt.float32)
        nc.sync.dma_start(b3_sb[:], b3c)
        gw_sb = cpool.tile([FD + 1, H], m_dt)
        nc.sync.dma_start(gw_sb[:], gw)
        bw_sb = cpool.tile([FD + 1, H], m_dt)
        nc.sync.dma_start(bw_sb[:], bw)
        domT_sb = cpool.tile([FD + 1, spc], m_dt)
        nc.sync.dma_start(domT_sb[:], dombT)
        out_sb = cpool.tile([1, spc], dt.float32)

        is_eq = mybir.AluOpType.is_equal

        # ---- PE warm-up: ~10us of dummy matmuls while DMA prefills, so HAM
        # flips to K=8/8 before the real stream starts and stays warm
        # through the pipeline-fill stalls of the first windows.
        warm_t = pm.tile([128, H], dt.float32, tag="pmlp")
        for i in range(100):
            nc.tensor.matmul(
                warm_t[:, 0:128], iotr_sb[:], iotr_sb[:],
                start=(i == 0), stop=(i == 99))

        GRP = 4  # windows per gamma/beta matmul group (N = GRP*128 <= 512)
        gbg = {}

        def emit_gamma_beta(wg):
            span = min(GRP * SEG_W, spc - wg * SEG_W)
            g_sbt = spool.tile([128, HC, span], m_dt, tag="gbg_g")
            b_sbt = spool.tile([128, HC, span], m_dt, tag="gbg_b")
            dom_s = domT_sb[:, wg * SEG_W: wg * SEG_W + span]
            for hc in range(HC):
                for wsb, dst in ((gw_sb, g_sbt), (bw_sb, b_sbt)):
                    pgb_t = pm.tile([128, H], dt.float32, tag="pmlp")
                    nc.tensor.matmul(
                        pgb_t[:, 0:span],
                        wsb[:, hc * 128:(hc + 1) * 128], dom_s,
                        start=True, stop=True)
                    nc.scalar.copy(dst[:, hc, :], pgb_t[:, 0:span])
            gbg[wg] = (g_sbt, b_sbt)

        nblocks = t_tiles // xt
        plan, order = _block_plan(nblocks)
        qblocks = _quad_blocks(nblocks, plan) if (narrow and quads) else ()
        GW = GRP * SEG_W  # batched MLP width (4 windows of segments)

        gmg = None
        for w in range(windows):
            if w % GRP == 0:
                emit_gamma_beta(w)
                gmg = spool.tile([128, HC, GW], m_dt, tag="gmg")
            # --- batch-relative ids for this window: [128 part, t_tiles] ---
            brt_sb = bpool.tile([128, t_tiles], dt.float32)
            nc.sync.dma_start(brt_sb[:], brt[w])

            # --- segment-sum for this window: accumulate [128 seg, H] ---
            # Narrow blocks use M=64 one-hots at segment base 0 or 64 (host
            # rebases the ids), halving the is_equal cost; the straddling
            # blocks stay M=128.  The first M=128 block runs first: its
            # initial matmul carries the start flag and initializes all 128
            # PSUM partitions.  Quad blocks hold groups of 4 same-segment
            # rows: the DVE pre-sums them 4:1 so the PE runs 2 matmuls
            # instead of 8.
            pg_t = pg.tile([128, H], dt.float32)
            base = w * npw

            def build_oh(cols, m, n):
                # One batched is_equal for n tiles: oh[p, c, s] =
                # (brt[p, cols+c] == iota[s]), via stride-0 broadcast APs.
                oh_blk = ohpool.tile([128, n, m], m_dt, tag=f"oh{m}x{n}")
                brt_sl = brt_sb[:, cols:cols + n]
                iot_sl = iotf_sb[:, 0:m]
                brt_b = bass.AP(brt_sl.tensor, brt_sl.offset,
                                list(brt_sl.ap) + [[0, m]])
                iot_b = bass.AP(iot_sl.tensor, iot_sl.offset,
                                [list(iot_sl.ap[0]), [0, n], list(iot_sl.ap[1])])
                nc.vector.tensor_tensor(oh_blk[:], brt_b, iot_b, is_eq)
                return oh_blk

            last_blk = order[-1] if narrow else nblocks - 1
            for bi, blk in enumerate(order if narrow else range(nblocks)):
                x_sb = xpool.tile([128, xt, H], x_dt)
                rows = xp[base + blk * xt * 128: base + (blk + 1) * xt * 128, :]
                # Partition p holds rows p*xt..(p+1)*xt of the block: each
                # partition line is one contiguous xt*H-byte read.
                nc.sync.dma_start(x_sb[:], rows.rearrange("(p c) h -> p c h", c=xt))
                m, sb = plan[blk] if narrow else (128, 0)
                st = bi == 0
                sp = blk == last_blk
                if blk in qblocks:
                    # Pair-sum c with c+1 (strided), then pairs 0,2 with 1,3:
                    # merged[:, mi, :] = sum of 4 same-segment source rows.
                    x_ap = x_sb[:]
                    ap0 = bass.AP(x_ap.tensor, x_ap.offset,
                                  [list(x_ap.ap[0]), [2 * H, xt // 2], [1, H]])
                    ap1 = bass.AP(x_ap.tensor, x_ap.offset + H,
                                  [list(x_ap.ap[0]), [2 * H, xt // 2], [1, H]])
                    pairs = qpool.tile([128, xt // 2, H], m_dt, tag="pairs")
                    nc.vector.tensor_tensor(pairs[:], ap0, ap1,
                                            mybir.AluOpType.add)
                    p_ap = pairs[:]
                    bp0 = bass.AP(p_ap.tensor, p_ap.offset,
                                  [list(p_ap.ap[0]), [2 * H, xt // 4], [1, H]])
                    bp1 = bass.AP(p_ap.tensor, p_ap.offset + H,
                                  [list(p_ap.ap[0]), [2 * H, xt // 4], [1, H]])
                    merged = qpool.tile([128, xt // 4, H], m_dt, tag="merged")
                    nc.vector.tensor_tensor(merged[:], bp0, bp1,
                                            mybir.AluOpType.add)
                    oh_blk = build_oh(blk * xt, m, xt // 4)
                    for mi in range(xt // 4):
                        nc.tensor.matmul(
                            pg_t[sb:sb + m, :], oh_blk[:, mi, :],
                            merged[:, mi, :],
                            start=(st and mi == 0),
                            stop=(sp and mi == xt // 4 - 1))
                else:
                    oh_blk = build_oh(blk * xt, m, xt)
                    for c in range(xt):
                        nc.tensor.matmul(
                            pg_t[sb:sb + m, :], oh_blk[:, c, :], x_sb[:, c, :],
                            start=(st and c == 0),
                            stop=(sp and c == xt - 1))

            # --- evict g to SBUF (bf16), transpose, FiLM into group buf ---
            g_sb = spool.tile([128, H], m_dt, tag="g")
            nc.scalar.copy(g_sb[:], pg_t[:])
            pt_t = pt.tile([128, H], m_dt)
            for hc in range(HC):
                nc.tensor.transpose(
                    pt_t[:, hc * 128:(hc + 1) * 128],
                    g_sb[:, hc * 128:(hc + 1) * 128],
                    iden_sb[:])
            pt_v = pt_t[:].rearrange("p (c s) -> p c s", c=HC)
            g_sbt, b_sbt = gbg[(w // GRP) * GRP]
            lo = (w % GRP) * SEG_W
            gm_v = gmg[:, :, lo:lo + SEG_W]
            nc.vector.tensor_mul(gm_v, pt_v, g_sbt[:, :, lo:lo + SEG_W])
            nc.vector.tensor_add(gm_v, gm_v, b_sbt[:, :, lo:lo + SEG_W])

            if w % GRP != GRP - 1:
                continue

            # --- batched MLP over GRP windows: N=512 matmuls ---
            wg = w - GRP + 1
            h1_sb = spool.tile([128, HC, GW], m_dt, tag="h1")
            for jc in range(JC):
                ph1 = pm.tile([128, GW], dt.float32, tag="pmlp")
                for hc in range(HC):
                    nc.tensor.matmul(
                        ph1[:], w1_sb[:, hc, jc * 128:(jc + 1) * 128],
                        gmg[:, hc, :], start=(hc == 0), stop=(hc == HC - 1))
                nc.scalar.activation(
                    h1_sb[:, jc, :], ph1[:],
                    mybir.ActivationFunctionType.Silu,
                    bias=b1_sb[:, jc:jc + 1])

            h2_sb = spool.tile([128, KC, GW], m_dt, tag="h2")
            for kc in range(KC):
                ph2 = pm.tile([128, GW], dt.float32, tag="pmlp")
                for hc in range(HC):
                    nc.tensor.matmul(
                        ph2[:], w2_sb[:, hc, kc * 128:(kc + 1) * 128],
                        h1_sb[:, hc, :], start=(hc == 0), stop=(hc == HC - 1))
                nc.scalar.activation(
                    h2_sb[:, kc, :], ph2[:],
                    mybir.ActivationFunctionType.Silu,
                    bias=b2_sb[:, kc:kc + 1])

            # --- output head: out[s] = sum_k w3[k] h2T[k, s] + b3 ---
            po = pm.tile([1, GW], dt.float32, tag="pmlp")
            for kc in range(KC):
                nc.tensor.matmul(
                    po[:], w3_sb[:, kc:kc + 1], h2_sb[:, kc, :],
                    start=(kc == 0), stop=(kc == KC - 1))
            nc.scalar.activation(
                out_sb[0:1, wg * SEG_W: wg * SEG_W + GW], po[:],
                mybir.ActivationFunctionType.Identity,
                bias=b3_sb[0:1, 0:1])

        nc.sync.dma_start(out, out_sb[:])

    nc.compile()
    return nc


def narrow_feasible(batch: np.ndarray, spc: int, t_tiles: int, xt: int,
                    n_cores: int) -> bool:
    """True iff every block's real segment ids fit its static narrow base."""
    windows = spc // SEG_W
    nblocks = t_tiles // xt
    plan, _ = _block_plan(nblocks)
    cap = xt * 128
    for core in range(n_cores):
        seg0 = core * spc
        w_starts = np.searchsorted(
            batch, seg0 + SEG_W * np.arange(windows + 1), side="left")
        for w in range(windows):
            s, e = int(w_starts[w]), int(w_starts[w + 1])
            ids = batch[s:e] - (seg0 + w * SEG_W)
            for b in range(nblocks):
                m, sb = plan[b]
                if m == 128:
                    continue
                blk = ids[b * cap:(b + 1) * cap]
                if len(blk) == 0:
                    continue
                if blk[0] < sb or blk[-1] > sb + m - 1:
                    return False
    return True


def _window_placement(ids: np.ndarray, nblocks: int, xt: int, plan, qblocks):
    """Map one window's (segment-sorted) rows to padded slot positions.

    Returns (place, brt_flat) -- place[i] is the flat slot of row i, and
    brt_flat the per-slot (rebased) segment ids -- or None if the static
    quad/narrow layout can't hold this window's data.

    Quad blocks hold groups of 4 same-segment rows at slots
    (p, mi*4 + j); their merged-tile segment ids live at columns mi of the
    block.  Leftover (non-quadded) rows go to the last block.
    """
    cap = xt * 128
    npw = nblocks * cap
    cnt = len(ids)
    offs = np.repeat(
        np.array([plan[b][1] for b in range(nblocks)], np.float32), cap)
    brt_flat = np.full(npw, -1.0e9, dtype=np.float32)
    place = np.empty(cnt, np.int64)

    if not qblocks:
        place[:] = np.arange(cnt)
        brt_flat[:cnt] = ids.astype(np.float32)
        brt_flat -= offs
        return place, brt_flat

    qb0 = qblocks[0]
    d_head = qb0 * cap
    n_head = min(cnt, d_head)
    place[:n_head] = np.arange(n_head)
    brt_flat[:n_head] = ids[:n_head].astype(np.float32)

    if cnt > d_head:
        rem = ids[d_head:]
        nrem = cnt - d_head
        need = len(qblocks) * cap
        segs_u, first_idx = np.unique(rem, return_index=True)
        counts = np.diff(np.append(first_idx, nrem))
        quadable = (counts // 4) * 4
        cumq = np.cumsum(quadable)
        prev = cumq - quadable
        take = np.clip(need - prev, 0, quadable)
        take = (take // 4) * 4
        if take.sum() < need:
            return None
        inv = np.searchsorted(segs_u, rem)
        r_within = np.arange(nrem) - first_idx[inv]
        qmask = r_within < take[inv]
        qrows = np.nonzero(qmask)[0]
        left = np.nonzero(~qmask)[0]
        if len(left) > cap:
            return None
        quad_segs = rem[qrows[0::4]]
        ml, sbl = plan[nblocks - 1]
        if len(left) and ml != 128 and (rem[left].min() < sbl
                                        or rem[left].max() > sbl + ml - 1):
            return None
        nqb = cap // 4  # quads per block
        q = np.arange(need // 4)
        blkq = np.asarray(qblocks)[q // nqb]
        for i, b in enumerate(qblocks):
            mq, sbq = plan[b]
            if mq == 128:
                continue
            qs = quad_segs[i * nqb:(i + 1) * nqb]
            if len(qs) and (qs.min() < sbq or qs.max() > sbq + mq - 1):
                return None
        qq = q % nqb
        mi = qq // 128
        p = qq % 128
        dest = blkq * cap + p * xt + mi * 4
        place[d_head + qrows] = (dest[:, None] + np.arange(4)[None, :]).reshape(-1)
        place[d_head + left] = (nblocks - 1) * cap + np.arange(len(left))
        brt_flat[place[d_head:]] = rem.astype(np.float32)
        # Merged-tile segment ids live at columns mi of each quad block
        # (overwriting the row-id scatter; columns >= xt//4 of quad blocks
        # are never read by the device's one-hot build).
        brt_flat[blkq * cap + p * xt + mi] = quad_segs.astype(np.float32)

    brt_flat -= offs
    return place, brt_flat


def quad_feasible(batch: np.ndarray, spc: int, t_tiles: int, xt: int,
                  n_cores: int) -> bool:
    """True iff the quad layout can hold every window's data."""
    windows = spc // SEG_W
    nblocks = t_tiles // xt
    plan, _ = _block_plan(nblocks)
    qblocks = _quad_blocks(nblocks, plan)
    if not qblocks:
        return False
    for core in range(n_cores):
        seg0 = core * spc
        w_starts = np.searchsorted(
            batch, seg0 + SEG_W * np.arange(windows + 1), side="left")
        for w in range(windows):
            s, e = int(w_starts[w]), int(w_starts[w + 1])
            ids = batch[s:e] - (seg0 + w * SEG_W)
            if _window_placement(ids, nblocks, xt, plan, qblocks) is None:
                return False
    return True


def prepare_core_inputs(
    x, batch, domain_emb, gamma_w, gamma_b, beta_w, beta_b,
    w1, b1, w2, b2, w3, b3,
    spc: int, t_tiles: int, xt: int, n_cores: int, mode: str = "quad",
):
    """Slice/pad/transpose the full inputs into one in_map per core."""
    windows = spc // SEG_W
    npw = SEG_W * t_tiles
    npad = windows * npw
    nblocks = t_tiles // xt
    cap = xt * 128
    if mode == "full":
        plan = [(128, 0)] * nblocks
        qblocks = ()
    else:
        plan, _ = _block_plan(nblocks)
        qblocks = _quad_blocks(nblocks, plan) if mode == "quad" else ()
    d_head = (qblocks[0] if qblocks else nblocks) * cap

    batch = np.ascontiguousarray(np.asarray(batch).astype(np.int64))
    x = np.asarray(x, dtype=np.float32)

    m_np = BF16
    shared = {
        "gw": np.ascontiguousarray(
            np.concatenate([np.asarray(gamma_w, np.float32).T,
                            np.asarray(gamma_b, np.float32)[None]], axis=0)).astype(m_np),
        "bw": np.ascontiguousarray(
            np.concatenate([np.asarray(beta_w, np.float32).T,
                            np.asarray(beta_b, np.float32)[None]], axis=0)).astype(m_np),
        "w1t": np.ascontiguousarray(np.asarray(w1, np.float32).T.astype(m_np)),
        "w2t": np.ascontiguousarray(np.asarray(w2, np.float32).T.astype(m_np)),
        "w3c": np.ascontiguousarray(
            np.asarray(w3, np.float32).reshape(H2 // 128, 128).T.astype(m_np)),
        "b1c": np.ascontiguousarray(np.asarray(b1, np.float32).reshape(H // 128, 128).T),
        "b2c": np.ascontiguousarray(np.asarray(b2, np.float32).reshape(H2 // 128, 128).T),
        "b3c": np.asarray(b3, np.float32).reshape(1, 1),
        "iden": np.eye(128, dtype=np.float32).astype(m_np),
        "iotr": np.tile(np.arange(128, dtype=np.float32), (128, 1)).astype(m_np),
        "iotf": np.tile(np.arange(128, dtype=np.float32), (128, 1)),
    }

    dom = np.asarray(domain_emb, np.float32)

    in_maps = []
    for core in range(n_cores):
        seg0 = core * spc
        w_starts = np.searchsorted(
            batch, seg0 + SEG_W * np.arange(windows + 1), side="left")
        xp_c = np.zeros((npad, H), dtype=F8E3)
        brt_c = np.full((windows, npw), -1.0e9, dtype=np.float32)
        for w in range(windows):
            s, e = int(w_starts[w]), int(w_starts[w + 1])
            cnt = e - s
            if cnt > npw:
                raise ValueError(f"window overflow: {cnt} > {npw}")
            if cnt == 0:
                continue
            ids = batch[s:e] - (seg0 + w * SEG_W)
            res = _window_placement(ids, nblocks, xt, plan, qblocks)
            assert res is not None, "placement infeasible (mode mismatch)"
            place, brt_flat = res
            xq = x[s:e].astype(F8E3)
            n_head = min(cnt, d_head)
            xp_c[w * npw: w * npw + n_head] = xq[:n_head]
            if cnt > n_head:
                xp_c[w * npw + place[n_head:]] = xq[n_head:]
            brt_c[w] = brt_flat
        # brt[w, p, blk*xt + c] must be the id of row blk*128*xt + p*xt + c
        # (partition-major contiguous x layout).
        brt_c = np.ascontiguousarray(
            brt_c.reshape(windows, nblocks, 128, xt)
                 .transpose(0, 2, 1, 3)
                 .reshape(windows, 128, t_tiles))
        dombT_c = np.ascontiguousarray(
            np.concatenate([dom[seg0:seg0 + spc].T,
                            np.ones((1, spc), np.float32)], axis=0)).astype(m_np)
        in_maps.append({"xp": xp_c, "brt": brt_c, "dombT": dombT_c, **shared})
    return in_maps


def _pick_t_tiles(batch: np.ndarray, spc: int, n_cores: int, xt: int) -> int:
    """Max padded tile count over all 128-segment windows, rounded to xt."""
    edges = np.arange(0, n_cores * spc + 1, SEG_W)
    starts = np.searchsorted(batch, edges, side="left")
    max_cnt = int(np.max(np.diff(starts))) if len(starts) > 1 else 0
    t = max(1, -(-max_cnt // 128))
    return -(-t // xt) * xt


_PROGRAM_CACHE: dict = {}

XT = 8  # node subtiles (of 128 rows) per x DMA

# Set by test harnesses: request an NTFF trace and stash the raw results.
TRACE = False
LAST_RESULT = None


def kernel(**inputs) -> np.ndarray:
    x = np.asarray(inputs["x"], dtype=np.float32)
    batch = np.ascontiguousarray(np.asarray(inputs["batch"]).astype(np.int64))
    assert x.shape == (N_NODES, H), x.shape

    spc = B_SEGS // N_CORES
    t_tiles = _pick_t_tiles(batch, spc, N_CORES, XT)
    if narrow_feasible(batch, spc, t_tiles, XT, N_CORES):
        mode = "quad" if quad_feasible(batch, spc, t_tiles, XT, N_CORES) \
            else "narrow"
    else:
        mode = "full"

    key = (spc, t_tiles, XT, N_CORES, mode)
    if key not in _PROGRAM_CACHE:
        _PROGRAM_CACHE[key] = build_program(
            spc, t_tiles, XT, N_CORES,
            narrow=(mode != "full"), quads=(mode == "quad"))
    nc = _PROGRAM_CACHE[key]

    in_maps = prepare_core_inputs(
        x, batch,
        inputs["domain_emb"], inputs["gamma_w"], inputs["gamma_b"],
        inputs["beta_w"], inputs["beta_b"],
        inputs["w1"], inputs["b1"], inputs["w2"], inputs["b2"],
        inputs["w3"], inputs["b3"],
        spc, t_tiles, XT, N_CORES, mode,
    )

    res = bass_utils.run_bass_kernel_spmd(
        nc, in_maps, core_ids=list(range(N_CORES)), trace=TRACE)
    global LAST_RESULT
    LAST_RESULT = res
    out = np.concatenate([res.results[c]["out"].reshape(-1) for c in range(N_CORES)])
    return np.ascontiguousarray(out.astype(np.float32))
